# revision 6
# baseline (speedup 1.0000x reference)
"""Trainium2 Bass kernel for nn_EnhancedOFTOutputLayer.

Math (per reference):
    S = 0.5*(A - A^T) per block (A = proj_R[b], 512x512, S skew-symmetric)
    Q = (I - S) @ inv(I + S + 1e-6 I)          (Cayley, orthogonal)
    filt = blockdiag(Q) @ weight               (block-row matmuls)
    y = x @ filt^T + bias

Sharding: tensor-parallel over the 8 blocks -> core b owns output rows
[512b, 512b+512).  x^T is replicated; each core computes
y_b^T = filt_b @ x^T  ([512, 8192]) with no cross-core communication.

Device algorithm per core:
    inv via Newton-Schulz: X <- X(2I - D X), X0 = 2I - D.  ||S||_2 ~ 0.64
    for this data regime so convergence is quadratic; 4 iterations reach
    the arithmetic floor.  All iterates are polynomials in S, so they
    commute and P(S)^T = P(-S); tracking (X, X^T) jointly lets every
    matmul use a stationary operand that is already transposed - no PE
    transposes anywhere.

Matmuls run in float32r (PE 1 cyc/row vs 4 for fp32; rel err ~3e-4,
far inside the 2e-2 gate).  PSUM accumulation is fp32.  fp32r rounding
happens in SWDGE cast-DMAs or DVE copies.  x ingestion is hybrid:
half the i-chunks stream via SWDGE cast-DMA, half via HWDGE fp32 +
DVE round, so neither DMA path limits the PE.

Host-side prep is layout-only: per-block slicing, transposes, and
re-tiling so every DMA reads one contiguous run per partition.
"""

import numpy as np

import concourse.bass as bass
import concourse.mybir as mybir
import concourse.tile as tile
from concourse import bacc
from concourse.bass_utils import run_bass_kernel_spmd

HID = 4096
NB = 8
BS = 512  # block size
NTOK = 8192  # 4*2048
P = 128
BC = BS // P  # 4 row-chunks per 512-mat
IC = HID // P  # 32 i-chunks
ICH = IC // 2  # i-chunks per ingestion path
TCH = 256  # token chunk (matmul moving free dim; fp32r needs >=256)
NT = NTOK // TCH
NEWTON_ITERS = 4
F32 = mybir.dt.float32
F32R = mybir.dt.float32r

_CACHE = {}


def _build():
    nc = bacc.Bacc("TRN2", target_bir_lowering=False)

    # all host-pretiled to [P, ...contiguous...] so DMAs are slab reads
    wb_d = nc.dram_tensor("wbl", [P, BC, HID], F32, kind="ExternalInput")
    pa_d = nc.dram_tensor("pal", [P, BC, BS], F32, kind="ExternalInput")
    pat_d = nc.dram_tensor("patl", [P, BC, BS], F32, kind="ExternalInput")
    eye_d = nc.dram_tensor("eyel", [P, BC, BS], F32, kind="ExternalInput")
    bias_d = nc.dram_tensor("bias2d", [P, BC], F32, kind="ExternalInput")
    xt_d = nc.dram_tensor("xtl", [NT, P, IC, TCH], F32, kind="ExternalInput")
    yt_d = nc.dram_tensor("ytl", [NT, P, BC, TCH], F32, kind="ExternalOutput")

    with tile.TileContext(nc) as tc:
        with tc.tile_pool(name="persist", bufs=1) as pp:
            filtT = pp.tile([P, IC, BS], F32R, tag="filtT")
            bias_sb = pp.tile([P, BC], F32, tag="bias")
            qt_sb = pp.tile([P, BC, BS], F32R, tag="qt")
            nc.sync.dma_start(bias_sb[:], bias_d[:])

            with (
                tc.tile_pool(name="cayley", bufs=1) as cp,
                tc.tile_pool(name="psA", bufs=4, space="PSUM") as psA,
            ):
                # fp32 inputs that only feed DVE (HWDGE, no cast: fast start)
                eye = cp.tile([P, BC, BS], F32, tag="eye")
                a_sb = cp.tile([P, BC, BS], F32, tag="t2", bufs=2)
                at_sb = cp.tile([P, BC, BS], F32, tag="t2t", bufs=2)
                nc.sync.dma_start(eye[:], eye_d[:])
                nc.sync.dma_start(a_sb[:], pa_d[:])
                nc.sync.dma_start(at_sb[:], pat_d[:])

                s_sb = cp.tile([P, BC, BS], F32, tag="s")
                dt_sb = cp.tile([P, BC, BS], F32R, tag="dt")  # D^T = I+S+e*I
                # S = 0.5*(A - A^T)
                nc.vector.tensor_sub(s_sb[:], a_sb[:], at_sb[:])
                nc.vector.tensor_scalar_mul(s_sb[:], s_sb[:], 0.5)
                ep = float(1.0 + 1e-6)
                em = float(1.0 - 1e-6)
                nc.vector.scalar_tensor_tensor(
                    dt_sb[:], eye[:], ep, s_sb[:],
                    mybir.AluOpType.mult, mybir.AluOpType.add)

                x_sb = cp.tile([P, BC, BS], F32R, tag="x", bufs=2)
                xt_sb = cp.tile([P, BC, BS], F32R, tag="xt", bufs=2)
                # X1 = 2I - D = I + S - eps*I  (first Newton step from X0=I)
                nc.vector.scalar_tensor_tensor(
                    x_sb[:], eye[:], em, s_sb[:],
                    mybir.AluOpType.mult, mybir.AluOpType.add)
                nc.vector.scalar_tensor_tensor(
                    xt_sb[:], eye[:], em, s_sb[:],
                    mybir.AluOpType.mult, mybir.AluOpType.subtract)

                def mm512(lhsT_tile, rhs_tile, out_sb, post=None):
                    # out = lhsT.T @ rhs for 512x512 mats in [P, BC, BS] tiles
                    for c in range(BC):
                        ps = psA.tile([P, BS], F32, tag="cay_ps")
                        for k in range(BC):
                            nc.tensor.matmul(
                                ps[:],
                                lhsT_tile[:, k, c * P:(c + 1) * P],
                                rhs_tile[:, k, :],
                                start=(k == 0),
                                stop=(k == BC - 1),
                            )
                        if post is None:
                            nc.vector.tensor_copy(out_sb[:, c, :], ps[:])
                        else:
                            post(c, ps)

                for it in range(NEWTON_ITERS):
                    t2 = cp.tile([P, BC, BS], F32R, tag="t2", bufs=2)
                    t2t = cp.tile([P, BC, BS], F32R, tag="t2t", bufs=2)

                    def post_t2(c, ps, _t2=t2):
                        # T2 = 2I - D@X  (DVE write rounds to fp32r)
                        nc.vector.scalar_tensor_tensor(
                            _t2[:, c, :], eye[:, c, :], 2.0, ps[:],
                            mybir.AluOpType.mult, mybir.AluOpType.subtract)

                    def post_t2t(c, ps, _t2t=t2t):
                        nc.vector.scalar_tensor_tensor(
                            _t2t[:, c, :], eye[:, c, :], 2.0, ps[:],
                            mybir.AluOpType.mult, mybir.AluOpType.subtract)

                    mm512(dt_sb, x_sb, None, post=post_t2)      # T1 = D@X
                    mm512(x_sb, dt_sb, None, post=post_t2t)     # T1t = (D@X)^T
                    xn = cp.tile([P, BC, BS], F32R, tag="x", bufs=2)
                    xnt = cp.tile([P, BC, BS], F32R, tag="xt", bufs=2)
                    mm512(xt_sb, t2, xn)        # Xn  = X @ T2
                    mm512(t2, xt_sb, xnt)       # Xnt = T2^T @ X^T
                    x_sb, xt_sb = xn, xnt

                # N^T = I - S, into a dead t2 slot (rounded by DVE write)
                nt_sb = cp.tile([P, BC, BS], F32R, tag="t2", bufs=2)
                nc.vector.tensor_sub(nt_sb[:], eye[:], s_sb[:])
                mm512(nt_sb, x_sb, qt_sb)       # Q^T = N @ X  (commute)

            # filt stage + big matmul share the stream pools so the x
            # prefetch overlaps the filt matmuls.
            with (
                tc.tile_pool(name="wstream", bufs=2) as wp,
                tc.tile_pool(name="xstream", bufs=2) as xp,
                tc.tile_pool(name="ystage", bufs=2) as yp,
                tc.tile_pool(name="psB", bufs=4, space="PSUM") as psB,
            ):
                # filt^T = W_b^T @ Q^T : lhsT = W_b (natural layout).
                # wb arrives fp32 via HWDGE; DVE rounds to fp32r.
                IGR = 4  # i-chunks per wb load group
                for g in range(IC // IGR):
                    wbt = wp.tile([P, BC, IGR * P], F32, tag="wb")
                    nc.sync.dma_start(
                        wbt[:], wb_d[:, :, g * IGR * P:(g + 1) * IGR * P])
                    wbr = wp.tile([P, BC, IGR * P], F32R, tag="wbr", bufs=1)
                    nc.vector.tensor_copy(wbr[:], wbt[:])
                    for ii in range(IGR):
                        i = g * IGR + ii
                        ps = psB.tile([P, BS], F32, tag="filt_ps", bufs=2)
                        for k in range(BC):
                            nc.tensor.matmul(
                                ps[:],
                                wbr[:, k, ii * P:(ii + 1) * P],
                                qt_sb[:, k, :],
                                start=(k == 0),
                                stop=(k == BC - 1),
                            )
                        nc.vector.tensor_copy(filtT[:, i, :], ps[:])

                # big matmul: y^T[o,t] = filt @ x^T, accumulate over i
                for t in range(NT):
                    xtt = xp.tile([P, IC, TCH], F32R, tag="xtile")
                    # first half: SWDGE cast-DMA rounds in flight
                    nc.gpsimd.dma_start(xtt[:, 0:ICH, :], xt_d[t, :, 0:ICH, :])
                    # second half: HWDGE fp32 + DVE round
                    xst = xp.tile([P, ICH, TCH], F32, tag="xstage")
                    nc.sync.dma_start(xst[:], xt_d[t, :, ICH:IC, :])
                    nc.vector.tensor_copy(xtt[:, ICH:IC, :], xst[:])
                    ys = yp.tile([P, BC, TCH], F32, tag="ys")
                    for o in range(BC):
                        ps = psB.tile([P, TCH], F32, tag="big_ps")
                        for i in range(IC):
                            nc.tensor.matmul(
                                ps[:],
                                filtT[:, i, o * P:(o + 1) * P],
                                xtt[:, i, :],
                                start=(i == 0),
                                stop=(i == IC - 1),
                            )
                        nc.scalar.activation(
                            ys[:, o, :], ps[:],
                            mybir.ActivationFunctionType.Identity,
                            bias=bias_sb[:, o:o + 1])
                    nc.sync.dma_start(yt_d[t], ys[:])

    nc.finalize()
    return nc


def kernel(weight, bias, x, proj_R, layer_idx=0, _trace=False, _tmpdir=None):
    weight = np.ascontiguousarray(np.asarray(weight, dtype=np.float32))
    bias = np.ascontiguousarray(np.asarray(bias, dtype=np.float32))
    x = np.ascontiguousarray(np.asarray(x, dtype=np.float32))
    proj_R = np.ascontiguousarray(np.asarray(proj_R, dtype=np.float32))

    if "nc" not in _CACHE:
        _CACHE["nc"] = _build()
    nc = _CACHE["nc"]

    def tile_pc(m):  # [BC*P, W] -> [P, BC, W] (partition-major tiling)
        return np.ascontiguousarray(
            m.reshape(BC, P, m.shape[1]).transpose(1, 0, 2))

    xt = x.reshape(NTOK, HID).T  # [HID, NTOK] view
    # [NT, P, IC, TCH]: xtl[t, p, c, j] = xt[c*P + p, t*TCH + j]
    xtl = np.ascontiguousarray(
        xt.reshape(IC, P, NT, TCH).transpose(2, 1, 0, 3))
    eye = tile_pc(np.eye(BS, dtype=np.float32))
    in_maps = []
    for b in range(NB):
        a = proj_R[b]
        in_maps.append({
            "wbl": tile_pc(weight[b * BS:(b + 1) * BS, :]),
            "pal": tile_pc(a),
            "patl": tile_pc(np.ascontiguousarray(a.T)),
            "eyel": eye,
            "bias2d": np.ascontiguousarray(
                bias[b * BS:(b + 1) * BS].reshape(BC, P).T),
            "xtl": xtl,
        })

    res = run_bass_kernel_spmd(nc, in_maps, core_ids=list(range(NB)),
                               trace=_trace, tmpdir=_tmpdir)
    out = np.empty((NTOK, HID), dtype=np.float32)
    for b in range(NB):
        # ytl[t, p, c, j] = y^T[c*P + p, t*TCH + j]
        ytb = np.ascontiguousarray(
            res.results[b]["ytl"].transpose(2, 1, 0, 3)).reshape(BS, NTOK)
        out[:, b * BS:(b + 1) * BS] = ytb.T
    if _trace:
        _CACHE["last_exec_time_ns"] = res.exec_time_ns
        _CACHE["last_results"] = res
    return out.reshape(4, 2048, HID)


# revision 8
# speedup vs baseline: 1.0789x; 1.0789x over previous
"""Trainium2 Bass kernel for nn_EnhancedOFTOutputLayer.

Math (per reference):
    S = 0.5*(A - A^T) per block (A = proj_R[b], 512x512, S skew-symmetric)
    Q = (I - S) @ inv(I + S + 1e-6 I)          (Cayley, orthogonal)
    filt = blockdiag(Q) @ weight               (block-row matmuls)
    y = x @ filt^T + bias

Sharding: tensor-parallel over the 8 blocks -> core b owns output rows
[512b, 512b+512).  x^T is replicated; each core computes
y_b^T = filt_b @ x^T  ([512, 8192]) with no cross-core communication.

Device algorithm per core:
    inv via Newton-Schulz: X <- X(2I - D X), X0 = 2I - D.  ||S||_2 ~ 0.64
    for this data regime so convergence is quadratic; 4 iterations reach
    the arithmetic floor.  All iterates are polynomials in S, so they
    commute and P(S)^T = P(-S); tracking (X, X^T) jointly lets every
    matmul use a stationary operand that is already transposed - no PE
    transposes anywhere.

Matmuls run in float32r (PE 1 cyc/row vs 4 for fp32; rel err ~3e-4,
far inside the 2e-2 gate).  PSUM accumulation is fp32.  fp32r rounding
happens in SWDGE cast-DMAs or DVE copies.  x ingestion is hybrid:
half the i-chunks stream via SWDGE cast-DMA, half via HWDGE fp32 +
DVE round, so neither DMA path limits the PE.

Host-side prep is layout-only: per-block slicing, transposes, and
re-tiling so every DMA reads one contiguous run per partition.
"""

import numpy as np

import concourse.bass as bass
import concourse.mybir as mybir
import concourse.tile as tile
from concourse import bacc
from concourse.bass_utils import run_bass_kernel_spmd

HID = 4096
NB = 8
BS = 512  # block size
NTOK = 8192  # 4*2048
P = 128
BC = BS // P  # 4 row-chunks per 512-mat
IC = HID // P  # 32 i-chunks
ICH = 28  # i-chunks via SWDGE cast-DMA; the rest via HWDGE + DVE round
TCH = 256  # token chunk (matmul moving free dim; fp32r needs >=256)
NT = NTOK // TCH
NEWTON_ITERS = 4
F32 = mybir.dt.float32
F32R = mybir.dt.float32r

_CACHE = {}


def _build():
    nc = bacc.Bacc("TRN2", target_bir_lowering=False)

    # all host-pretiled to [P, ...contiguous...] so DMAs are slab reads
    wb_d = nc.dram_tensor("wbl", [P, BC, HID], F32, kind="ExternalInput")
    pa_d = nc.dram_tensor("pal", [P, BC, BS], F32, kind="ExternalInput")
    pat_d = nc.dram_tensor("patl", [P, BC, BS], F32, kind="ExternalInput")
    eye_d = nc.dram_tensor("eyel", [P, BC, BS], F32, kind="ExternalInput")
    bias_d = nc.dram_tensor("bias2d", [P, BC], F32, kind="ExternalInput")
    xt_d = nc.dram_tensor("xtl", [NT, P, IC, TCH], F32, kind="ExternalInput")
    yt_d = nc.dram_tensor("ytl", [NT, P, BC, TCH], F32, kind="ExternalOutput")

    with tile.TileContext(nc) as tc:
        with tc.tile_pool(name="persist", bufs=1) as pp:
            filtT = pp.tile([P, IC, BS], F32R, tag="filtT")
            bias_sb = pp.tile([P, BC], F32, tag="bias")
            qt_sb = pp.tile([P, BC, BS], F32R, tag="qt")
            nc.sync.dma_start(bias_sb[:], bias_d[:])

            with (
                tc.tile_pool(name="cayley", bufs=1) as cp,
                tc.tile_pool(name="psA", bufs=4, space="PSUM") as psA,
            ):
                # fp32 inputs that only feed DVE (HWDGE, no cast: fast start)
                eye = cp.tile([P, BC, BS], F32, tag="eye")
                a_sb = cp.tile([P, BC, BS], F32, tag="t2", bufs=2)
                at_sb = cp.tile([P, BC, BS], F32, tag="t2t", bufs=2)
                nc.sync.dma_start(eye[:], eye_d[:])
                nc.sync.dma_start(a_sb[:], pa_d[:])
                nc.sync.dma_start(at_sb[:], pat_d[:])

                s_sb = cp.tile([P, BC, BS], F32, tag="s")
                dt_sb = cp.tile([P, BC, BS], F32R, tag="dt")  # D^T = I+S+e*I
                # S = 0.5*(A - A^T)
                nc.vector.tensor_sub(s_sb[:], a_sb[:], at_sb[:])
                nc.vector.tensor_scalar_mul(s_sb[:], s_sb[:], 0.5)
                ep = float(1.0 + 1e-6)
                em = float(1.0 - 1e-6)
                nc.vector.scalar_tensor_tensor(
                    dt_sb[:], eye[:], ep, s_sb[:],
                    mybir.AluOpType.mult, mybir.AluOpType.add)

                x_sb = cp.tile([P, BC, BS], F32R, tag="x", bufs=2)
                xt_sb = cp.tile([P, BC, BS], F32R, tag="xt", bufs=2)
                # X1 = 2I - D = I + S - eps*I  (first Newton step from X0=I)
                nc.vector.scalar_tensor_tensor(
                    x_sb[:], eye[:], em, s_sb[:],
                    mybir.AluOpType.mult, mybir.AluOpType.add)
                nc.vector.scalar_tensor_tensor(
                    xt_sb[:], eye[:], em, s_sb[:],
                    mybir.AluOpType.mult, mybir.AluOpType.subtract)

                def mm512(lhsT_tile, rhs_tile, out_sb, post=None):
                    # out = lhsT.T @ rhs for 512x512 mats in [P, BC, BS] tiles
                    for c in range(BC):
                        ps = psA.tile([P, BS], F32, tag="cay_ps")
                        for k in range(BC):
                            nc.tensor.matmul(
                                ps[:],
                                lhsT_tile[:, k, c * P:(c + 1) * P],
                                rhs_tile[:, k, :],
                                start=(k == 0),
                                stop=(k == BC - 1),
                            )
                        if post is None:
                            nc.vector.tensor_copy(out_sb[:, c, :], ps[:])
                        else:
                            post(c, ps)

                for it in range(NEWTON_ITERS):
                    t2 = cp.tile([P, BC, BS], F32R, tag="t2", bufs=2)
                    t2t = cp.tile([P, BC, BS], F32R, tag="t2t", bufs=2)

                    def post_t2(c, ps, _t2=t2):
                        # T2 = 2I - D@X  (DVE write rounds to fp32r)
                        nc.vector.scalar_tensor_tensor(
                            _t2[:, c, :], eye[:, c, :], 2.0, ps[:],
                            mybir.AluOpType.mult, mybir.AluOpType.subtract)

                    def post_t2t(c, ps, _t2t=t2t):
                        nc.vector.scalar_tensor_tensor(
                            _t2t[:, c, :], eye[:, c, :], 2.0, ps[:],
                            mybir.AluOpType.mult, mybir.AluOpType.subtract)

                    mm512(dt_sb, x_sb, None, post=post_t2)      # T1 = D@X
                    mm512(x_sb, dt_sb, None, post=post_t2t)     # T1t = (D@X)^T
                    xn = cp.tile([P, BC, BS], F32R, tag="x", bufs=2)
                    xnt = cp.tile([P, BC, BS], F32R, tag="xt", bufs=2)
                    mm512(xt_sb, t2, xn)        # Xn  = X @ T2
                    mm512(t2, xt_sb, xnt)       # Xnt = T2^T @ X^T
                    x_sb, xt_sb = xn, xnt

                # N^T = I - S, into a dead t2 slot (rounded by DVE write)
                nt_sb = cp.tile([P, BC, BS], F32R, tag="t2", bufs=2)
                nc.vector.tensor_sub(nt_sb[:], eye[:], s_sb[:])
                mm512(nt_sb, x_sb, qt_sb)       # Q^T = N @ X  (commute)

            # filt stage + big matmul share the stream pools so the x
            # prefetch overlaps the filt matmuls.
            with (
                tc.tile_pool(name="wstream", bufs=2) as wp,
                tc.tile_pool(name="xstream", bufs=2) as xp,
                tc.tile_pool(name="ystage", bufs=2) as yp,
                tc.tile_pool(name="psB", bufs=4, space="PSUM") as psB,
            ):
                # filt^T = W_b^T @ Q^T : lhsT = W_b (natural layout).
                # wb arrives fp32 via HWDGE; DVE rounds to fp32r.
                IGR = 4  # i-chunks per wb load group
                for g in range(IC // IGR):
                    wbt = wp.tile([P, BC, IGR * P], F32, tag="wb")
                    nc.sync.dma_start(
                        wbt[:], wb_d[:, :, g * IGR * P:(g + 1) * IGR * P])
                    wbr = wp.tile([P, BC, IGR * P], F32R, tag="wbr", bufs=1)
                    nc.vector.tensor_copy(wbr[:], wbt[:])
                    for ii in range(IGR):
                        i = g * IGR + ii
                        ps = psB.tile([P, BS], F32, tag="filt_ps", bufs=2)
                        for k in range(BC):
                            nc.tensor.matmul(
                                ps[:],
                                wbr[:, k, ii * P:(ii + 1) * P],
                                qt_sb[:, k, :],
                                start=(k == 0),
                                stop=(k == BC - 1),
                            )
                        nc.vector.tensor_copy(filtT[:, i, :], ps[:])

                # big matmul: y^T[o,t] = filt @ x^T, accumulate over i
                for t in range(NT):
                    xtt = xp.tile([P, IC, TCH], F32R, tag="xtile")
                    # most chunks: SWDGE cast-DMA rounds in flight
                    nc.gpsimd.dma_start(xtt[:, 0:ICH, :], xt_d[t, :, 0:ICH, :])
                    # remainder: HWDGE fp32 + DVE round (balances DMA paths)
                    xst = xp.tile([P, IC - ICH, TCH], F32, tag="xstage")
                    nc.sync.dma_start(xst[:], xt_d[t, :, ICH:IC, :])
                    nc.vector.tensor_copy(xtt[:, ICH:IC, :], xst[:])
                    ys = yp.tile([P, BC, TCH], F32, tag="ys")
                    for o in range(BC):
                        ps = psB.tile([P, TCH], F32, tag="big_ps")
                        for i in range(IC):
                            nc.tensor.matmul(
                                ps[:],
                                filtT[:, i, o * P:(o + 1) * P],
                                xtt[:, i, :],
                                start=(i == 0),
                                stop=(i == IC - 1),
                            )
                        nc.scalar.activation(
                            ys[:, o, :], ps[:],
                            mybir.ActivationFunctionType.Identity,
                            bias=bias_sb[:, o:o + 1])
                    nc.sync.dma_start(yt_d[t], ys[:])

    nc.finalize()
    return nc


def kernel(weight, bias, x, proj_R, layer_idx=0, _trace=False, _tmpdir=None):
    weight = np.ascontiguousarray(np.asarray(weight, dtype=np.float32))
    bias = np.ascontiguousarray(np.asarray(bias, dtype=np.float32))
    x = np.ascontiguousarray(np.asarray(x, dtype=np.float32))
    proj_R = np.ascontiguousarray(np.asarray(proj_R, dtype=np.float32))

    if "nc" not in _CACHE:
        _CACHE["nc"] = _build()
    nc = _CACHE["nc"]

    def tile_pc(m):  # [BC*P, W] -> [P, BC, W] (partition-major tiling)
        return np.ascontiguousarray(
            m.reshape(BC, P, m.shape[1]).transpose(1, 0, 2))

    xt = x.reshape(NTOK, HID).T  # [HID, NTOK] view
    # [NT, P, IC, TCH]: xtl[t, p, c, j] = xt[c*P + p, t*TCH + j]
    xtl = np.ascontiguousarray(
        xt.reshape(IC, P, NT, TCH).transpose(2, 1, 0, 3))
    eye = tile_pc(np.eye(BS, dtype=np.float32))
    in_maps = []
    for b in range(NB):
        a = proj_R[b]
        in_maps.append({
            "wbl": tile_pc(weight[b * BS:(b + 1) * BS, :]),
            "pal": tile_pc(a),
            "patl": tile_pc(np.ascontiguousarray(a.T)),
            "eyel": eye,
            "bias2d": np.ascontiguousarray(
                bias[b * BS:(b + 1) * BS].reshape(BC, P).T),
            "xtl": xtl,
        })

    res = run_bass_kernel_spmd(nc, in_maps, core_ids=list(range(NB)),
                               trace=_trace, tmpdir=_tmpdir)
    out = np.empty((NTOK, HID), dtype=np.float32)
    for b in range(NB):
        # ytl[t, p, c, j] = y^T[c*P + p, t*TCH + j]
        ytb = np.ascontiguousarray(
            res.results[b]["ytl"].transpose(2, 1, 0, 3)).reshape(BS, NTOK)
        out[:, b * BS:(b + 1) * BS] = ytb.T
    if _trace:
        _CACHE["last_exec_time_ns"] = res.exec_time_ns
        _CACHE["last_results"] = res
    return out.reshape(4, 2048, HID)
